# revision 41
# baseline (speedup 1.0000x reference)
"""Two-layer GAT forward on 8 Trainium2 NeuronCores.

Strategy: edges partitioned by destination node across the 8 cores (1250
dst nodes per core), sorted by destination.  Node features are replicated
for the layer-1 GEMM (bf16 compute), whose output rows are stored in fp8
(e3m4) with bf16 attention scores packed in the same 1280B row, in a
per-device ROLLED node order so each core's local nodes occupy rows
0..1249 (windows align with 128-row GEMM tiles; the window's dst scores
are copied out of the GEMM epilogue directly, no gather needed).

Per-edge source rows are fetched with one dma_gather per group; the
segment softmax + scatter-add run as one-hot segment matmuls on the
TensorEngine accumulating in PSUM per 128-dst window.  The dst-major
one-hot (for per-edge dst scores) is built from a host-replicated dstrep
tensor with a single is_equal per group.  The layer-2 GEMM is fused into
the same window loop; the AllGather of layer-2 rows is split in two so
the first half overlaps the remaining windows.  Layer-2 aggregation
mirrors layer 1 with 512B bf16 rows.

Self-contained: hardcodes the problem shapes from the spec.
"""
import os
import sys
import numpy as np

try:
    import concourse.bass as bass  # noqa
except ImportError:
    sys.path.insert(0, "/opt/trn_rl_repo")

import concourse.bass as bass
import concourse.tile as tile
from concourse import mybir, bacc
from concourse.bass_utils import run_bass_kernel_spmd

# ---------------------------------------------------------------- problem dims
N, E = 10000, 160000
FIN, H1, C1, C2 = 256, 8, 128, 128
D1 = H1 * C1  # 1024
SLOPE = 0.2
NDEV = 8
NLOC = N // NDEV  # 1250
NW = (NLOC + 127) // 128  # 10 windows of 128 dsts (last = 98)
LASTW = NLOC - (NW - 1) * 128  # 98
NT = (N + 127) // 128  # 79 node tiles for the replicated GEMM
SPLIT = 640  # windows 0-4 -> first AllGather
REST = NLOC - SPLIT  # 610

F32 = mybir.dt.float32
BF16 = mybir.dt.bfloat16
FP8 = mybir.dt.float8e3
I16 = mybir.dt.int16
ALU = mybir.AluOpType
ACTF = mybir.ActivationFunctionType

# hcat row (fp8 bytes): [ h 1024 | s_src 8xbf16 (bf16 slots 512:520) | pad ]
ROW1 = 1280
# layer-2 row (bf16): [ h2 128 | s2src f32 (f32 slot 64) | pad ] -> 256 = 512B
ROW2 = 256

_EPS = 1e-30

# --------------------------------------------------------------------- patches


def _apply_drain_patch():
    """This walrus build rejects >1 sync-wait on the Tile-exit Drain; split the
    waits across consecutive drains (semantically identical)."""
    from concourse.vector_clock import ScopedClock

    def _patched(self, tick_clock, wait_clock):
        drain_inst = self.nc.sync.drain()
        wait_clock.add_sem_waits(
            drain_inst.ins, ScopedClock({None: tick_clock.global_clock})
        )
        si = drain_inst.ins.sync_info
        if si is not None and len(si.on_wait) > 1:
            waits = list(si.on_wait)
            si.on_wait = waits[:1]
            drain_inst.ins.sync_info = si
            for i in range(1, len(waits)):
                extra = self.nc.sync.drain()
                esi = extra.ins.sync_info
                if esi is None:
                    esi = mybir.SyncInfo(on_wait=[], on_update=[])
                esi.on_wait = list(esi.on_wait) + waits[i : i + 1]
                extra.ins.sync_info = esi
        self.nc.all_engine_barrier()
        assert self.sems is not None
        popped = self.nc._tile_sem_poison_stack.pop()
        assert popped is self._sem_poison
        self.nc.clear_and_free_semaphores(list(self.sems.allocated().values()))
        self.nc.all_engine_barrier()

    tile.TileContext._drain_and_barrier = _patched


_apply_drain_patch()


def _fix_prep_sems(nc):
    """PREPARE_ONLY SWDGE preps bake the user-supplied sem into their DMA
    descriptors, but Tile's wait pass makes data consumers wait on the DMASW
    lane sems it assigned (assuming the descriptors bump those).  Re-point
    each prep's descriptor sem at its assigned DMASW lane sem (exact lane
    from bass_scheduled_proc, scope suffix from the Pool engine-sem entry).
    Per-engine in-order ring draining makes the lane-merged waits sound."""
    from concourse.tile_sem_assignment import PROC_NAME_TO_IDX

    idx2name = {v: k for k, v in PROC_NAME_TO_IDX.items()}
    fn = nc.m.functions[0]
    name2id = {}
    for b in fn.blocks:
        for ins in b.instructions:
            si = ins.sync_info
            if si is None:
                continue
            for x in list(si.on_wait) + list(si.on_update):
                if getattr(x, "ant_name", None):
                    name2id[x.ant_name] = x.id
    nfix = 0
    for b in fn.blocks:
        for ins in b.instructions:
            if type(ins).__name__ != "InstDMAGatherAnt":
                continue
            if getattr(ins, "gen_mode", 0) != 1:
                continue
            lane = idx2name[ins.bass_scheduled_proc]
            assert lane.startswith("DMASW"), lane
            si = ins.sync_info
            ups = list(si.on_update)
            scope = None
            for x in ups:
                if x.ant_name.startswith("Pool_"):
                    scope = x.ant_name.split("_", 1)[1]
            assert scope is not None, ups
            target = f"{lane}_{scope}"
            assert target in name2id, (target, sorted(name2id))
            hit = False
            for x in ups:
                if x.ant_name == "gsem":
                    x.id = name2id[target]
                    x.ant_name = target
                    hit = True
            assert hit, ups
            si.on_update = ups
            ins.sync_info = si
            nfix += 1
    assert nfix > 0
    # Tile's one-wait-per-instruction merge can under-synchronize async
    # preps.  Conservatively raise every DMASW wait to the lane's cumulative
    # count as of the last earlier prep in scheduled order (triggers are
    # unconditional, so raised waits always satisfy eventually).
    cum = {}
    for b in fn.blocks:
        for ins in b.instructions:
            si = ins.sync_info
            if si is not None and si.on_wait:
                ws = list(si.on_wait)
                changed = False
                for x in ws:
                    c = cum.get(x.ant_name)
                    if (
                        x.ant_name.startswith("DMASW")
                        and c is not None
                        and x.wait_value is not None
                        and c > x.wait_value
                    ):
                        x.wait_value = c
                        changed = True
                if changed:
                    si.on_wait = ws
                    ins.sync_info = si
            if (
                type(ins).__name__ == "InstDMAGatherAnt"
                and getattr(ins, "gen_mode", 0) == 1
            ):
                upd = ins.sync_info.on_update[0]
                assert upd.ant_name.startswith("DMASW"), upd
                cum[upd.ant_name] = cum.get(upd.ant_name, 0) + upd.update_value
    return nfix


# ------------------------------------------------------------------- host prep


def _wrap_idx(idx):
    """dma_gather index layout: idx i at partition i%16, col i//16, replicated
    8x across the 128 partitions."""
    a = np.ascontiguousarray(idx.astype(np.int16).reshape(-1, 16).T)
    return np.ascontiguousarray(np.tile(a, (8, 1)))


def _bf(a):
    import ml_dtypes

    return np.ascontiguousarray(a).astype(ml_dtypes.bfloat16)


def _prep_edges(edge_index):
    """Shard edges by dst across devices, sort by dst, pad each 128-dst
    window to a uniform chunk count.  Self-loop edges are handled exactly
    on-device from local activations, so they are excluded here."""
    src = np.asarray(edge_index[0], np.int64)
    dst = np.asarray(edge_index[1], np.int64)

    per_dev = []
    max_chunks = 1
    for d in range(NDEV):
        base = d * NLOC
        sel = (dst >= base) & (dst < base + NLOC)
        s_d, t_d = src[sel], dst[sel]
        order = np.argsort(t_d, kind="stable")
        s_d, t_d = s_d[order], t_d[order]
        wid = (t_d - base) // 128
        cnts = np.bincount(wid, minlength=NW)
        max_chunks = max(max_chunks, int(np.max((cnts + 127) // 128)))
        per_dev.append((s_d, t_d, cnts))

    G = (max_chunks + 1) // 2  # chunks per gather group
    CPW = 2 * G  # uniform chunks per window
    TOT = NW * CPW * 128  # padded edges per device
    NCHUNK = NW * CPW

    devs = []
    for d in range(NDEV):
        base = d * NLOC
        s_d, t_d, cnts = per_dev[d]
        srcp = np.zeros(TOT, np.int64)
        dloc = np.full(TOT, -1.0, np.float32)
        starts = np.concatenate([[0], np.cumsum(cnts)])
        for w in range(NW):
            a, b = starts[w], starts[w + 1]
            o = w * CPW * 128
            n = b - a
            srcp[o : o + n] = s_d[a:b]
            dloc[o : o + n] = (t_d[a:b] - base - w * 128).astype(np.float32)
        # layer-1 gather rows: hcat is rolled so local nodes are rows 0..NLOC
        src1 = (srcp - base) % N
        # layer-2 gather rows: h2all is laid out [AG1: 8x640 | AG2: 8x610]
        if AGSPLIT:
            r, l = srcp // NLOC, srcp % NLOC
            src2 = np.where(
                l < SPLIT, r * SPLIT + l, NDEV * SPLIT + r * REST + (l - SPLIT)
            )
        else:
            src2 = srcp
        devs.append(
            {
                "srcidx": _wrap_idx(src1),
                "srcidx2": _wrap_idx(src2),
                "dstloc": _bf(dloc.reshape(NCHUNK, 128).T),
                "dstrep": _bf(np.tile(dloc[None, :], (128, 1))),
            }
        )
    return devs, G, CPW


# -------------------------------------------------------------- program build

_CACHE = {}

AGSPLIT = os.environ.get("KAGSPLIT", "1") == "1"


def _build(G, CPW, add_b1, add_b2):
    NCHUNK = NW * CPW
    TOT = NCHUNK * 128

    nc = bacc.Bacc()
    dp = nc.declare_dram_parameter
    # per-device inputs (xT is rolled per device)
    xT_d = dp("xT", [FIN, N], BF16, isOutput=False)
    srcidx_d = dp("srcidx", [128, TOT // 16], I16, isOutput=False)
    srcidx2_d = dp("srcidx2", [128, TOT // 16], I16, isOutput=False)
    dstloc_d = dp("dstloc", [128, NCHUNK], BF16, isOutput=False)
    dstrep_d = dp("dstrep", [128, TOT], BF16, isOutput=False)
    # shared inputs
    W1_d = dp("W1aug", [FIN, D1 + 16], BF16, isOutput=False)
    W2_d = dp("W2aug", [D1, C2 + 2], BF16, isOutput=False)
    iota_d = dp("iota_rep", [128, G * 128], BF16, isOutput=False)
    iotac_d = dp("iota_col", [128, 1], F32, isOutput=False)
    ident_d = dp("ident", [128, 128], BF16, isOutput=False)
    b1_d = dp("b1bc", [128, D1], F32, isOutput=False)
    b2_d = dp("b2bc", [128, C2], F32, isOutput=False)
    # output
    out_d = dp("out", [NLOC, C2], F32, isOutput=True)
    # internal DRAM
    hcat = nc.dram_tensor("hcat", [N, ROW1], FP8)
    h2loc = nc.dram_tensor("h2loc", [NLOC, ROW2], BF16)
    h2all = nc.dram_tensor("h2all", [N, ROW2], BF16, addr_space="Shared")

    with tile.TileContext(nc) as tc:
        with tc.tile_pool(name="const", bufs=1) as constp:
            iota_t = constp.tile([128, G * 128], BF16)
            nc.sync.dma_start(iota_t[:], iota_d[:])
            iotac_t = constp.tile([128, 1], F32)
            nc.sync.dma_start(iotac_t[:], iotac_d[:])
            ident_t = constp.tile([128, 128], BF16)
            nc.sync.dma_start(ident_t[:], ident_d[:])
            b1_t = b2_t = None
            if add_b1:
                b1_t = constp.tile([128, D1], F32)
                nc.sync.dma_start(b1_t[:], b1_d[:])
            if add_b2:
                b2_t = constp.tile([128, C2], F32)
                nc.sync.dma_start(b2_t[:], b2_d[:])
            srcidx_t = constp.tile([128, TOT // 16], I16)
            nc.sync.dma_start(srcidx_t[:], srcidx_d[:])
            srcidx2_t = constp.tile([128, TOT // 16], I16)
            nc.sync.dma_start(srcidx2_t[:], srcidx2_d[:])
            dstloc_t = constp.tile([128, NCHUNK], BF16)
            nc.sync.dma_start(dstloc_t[:], dstloc_d[:])
            W2_t = constp.tile([128, 8, C2 + 2], BF16)
            nc.sync.dma_start(
                W2_t[:], W2_d[:].rearrange("(k p) f -> p k f", p=128)
            )
            h1T_t = constp.tile([128, 8, NW * 128], BF16)
            sdw_t = constp.tile([128, NW, 8], BF16)
            ssw_t = constp.tile([128, NW, 8], BF16)
            hloc_t = constp.tile([128, NW, D1], BF16)
            h2loc_t = constp.tile([128, NW, C2], BF16)
            s2dcol_t = constp.tile([128, NW], BF16)
            nc.vector.memset(s2dcol_t[:], 0.0)
            s2scol_t = constp.tile([128, NW], BF16)
            nc.vector.memset(s2scol_t[:], 0.0)
            nc.vector.memset(h2loc_t[:], 0.0)

            # ---------------- Phase A: replicated h = x @ W1aug (rolled order)
            with (
                tc.tile_pool(name="gemmA", bufs=1) as gA,
                tc.tile_pool(name="outA", bufs=3) as oA,
                tc.tile_pool(name="psA", bufs=3, space="PSUM") as psA_p,
                tc.tile_pool(name="psAs", bufs=2, space="PSUM") as psAs_p,
            ):
                xT_t = gA.tile([128, 2, N], BF16)
                xTr = xT_d[:].rearrange("(k p) n -> p k n", p=128)
                nc.sync.dma_start(xT_t[:, :, 0:2560], xTr[:, :, 0:2560])
                nc.sync.dma_start(xT_t[:, :, 2560:N], xTr[:, :, 2560:N])
                W1_t = gA.tile([128, 2, D1 + 16], BF16)
                nc.sync.dma_start(
                    W1_t[:], W1_d[:].rearrange("(k p) f -> p k f", p=128)
                )
                for t in range(NT):
                    tl = min(128, N - t * 128)
                    ps = psA_p.tile([128, 1024], F32, tag="psA")
                    pss = psAs_p.tile([128, 16], F32, tag="psAs")
                    for k in range(2):
                        lhsT = xT_t[:, k, t * 128 : t * 128 + tl]
                        nc.tensor.matmul(
                            ps[:tl, 0:512], lhsT, W1_t[:, k, 0:512],
                            start=(k == 0), stop=(k == 1),
                        )
                        nc.tensor.matmul(
                            ps[:tl, 512:1024], lhsT, W1_t[:, k, 512:1024],
                            start=(k == 0), stop=(k == 1),
                        )
                        nc.tensor.matmul(
                            pss[:tl, 0:16], lhsT, W1_t[:, k, 1024:1040],
                            start=(k == 0), stop=(k == 1),
                        )
                    hc = oA.tile([128, ROW1], FP8, tag="hc")
                    if t < 3:
                        nc.vector.memset(hc[:, 1040:1280], 0.0)
                    nc.vector.tensor_copy(hc[:tl, 0:512], ps[:tl, 0:512])
                    nc.scalar.activation(
                        hc[:tl, 512:1024], ps[:tl, 512:1024], ACTF.Copy
                    )
                    nc.vector.tensor_copy(
                        hc[:tl].bitcast(BF16)[:, 512:520], pss[:tl, 0:8]
                    )
                    if t < NW:
                        nc.vector.tensor_copy(sdw_t[:, t, :], pss[:, 8:16])
                        nc.vector.tensor_copy(ssw_t[:, t, :], pss[:, 0:8])
                        nc.vector.tensor_copy(
                            hloc_t[:, t, 0:512], ps[:, 0:512]
                        )
                        nc.scalar.activation(
                            hloc_t[:, t, 512:1024], ps[:, 512:1024], ACTF.Copy
                        )
                    nc.sync.dma_start(
                        hcat[t * 128 : t * 128 + tl, :], hc[:tl, :]
                    )

            # ---------------- Phase B+C: layer-1 aggregation + layer-2 GEMM
            def _bc_window(w, eB, hp, wB, oC, psw_p, den_p, es_p, psC_p):
                    wl = 128 if w < NW - 1 else LASTW
                    psw = psw_p.tile([128, 1024], F32, tag="psw")
                    den = den_p.tile([128, 8], F32, tag="den")
                    dsw = wB.tile([128, CPW * 128], BF16, tag="dsw")
                    nc.sync.dma_start(
                        dsw[:], dstrep_d[:, w * CPW * 128 : (w + 1) * CPW * 128]
                    )
                    for g in range(2):
                        k0 = w * CPW + g * G
                        o16 = k0 * 8
                        hg = eB.tile([128, G, ROW1], FP8, tag="hg")
                        gh = (G + 1) // 2
                        for hv in range(2):
                            c0, c1 = hv * gh, min(G, (hv + 1) * gh)
                            if c0 >= c1:
                                continue
                            nc.gpsimd.dma_gather(
                                out_ap=hg[:, c0:c1, :], in_ap=hcat[:, :],
                                idxs_ap=srcidx_t[:, o16 + c0 * 8 : o16 + c1 * 8],
                                num_idxs=(c1 - c0) * 128,
                                num_idxs_reg=(c1 - c0) * 128,
                                elem_size=ROW1, single_packet=True,
                            )
                        # dst-major one-hot + per-edge dst scores
                        Ssb = wB.tile([128, G * 128], BF16, tag="Ssb")
                        nc.vector.tensor_scalar(
                            Ssb[:], dsw[:, g * G * 128 : (g + 1) * G * 128],
                            iotac_t[:, 0:1], None, ALU.is_equal,
                        )
                        esp = es_p.tile([128, G * 8], F32, tag="esp")
                        for c in range(G):
                            nc.tensor.matmul(
                                esp[:, c * 8 : c * 8 + 8],
                                Ssb[:, c * 128 : (c + 1) * 128],
                                sdw_t[:, w, :], start=True, stop=True,
                            )
                        # p = exp(leaky_relu(ssrc + sdst))  [128, G, 8]
                        pt = eB.tile([128, G, 8], F32, tag="pt")
                        nc.vector.tensor_tensor(
                            pt[:],
                            hg[:].bitcast(BF16)[:, :, 512:520],
                            esp[:].rearrange("e (g h) -> e g h", h=8),
                            ALU.add,
                        )
                        lr = eB.tile([128, G, 8], F32, tag="lr")
                        nc.vector.scalar_tensor_tensor(
                            lr[:], pt[:], SLOPE, pt[:], ALU.mult, ALU.max
                        )
                        pb = eB.tile([128, G, 8], BF16, tag="pb")
                        nc.scalar.activation(pb[:], lr[:], ACTF.Exp)
                        # hgp = h * p (per-head broadcast), fp8 -> bf16
                        hgp = hp.tile([128, G, 8, C1], BF16, tag="hgp")
                        nc.vector.tensor_tensor(
                            hgp[:],
                            hg[:, :, 0:1024].rearrange(
                                "e g (h c) -> e g h c", c=C1
                            ),
                            pb[:].unsqueeze(3).broadcast_to([128, G, 8, C1]),
                            ALU.mult,
                        )
                        # e-major one-hot, batched over the group
                        stg = wB.tile([128, G * 128], BF16, tag="stg")
                        nc.vector.tensor_tensor(
                            stg[:].rearrange("e (g d) -> e g d", d=128),
                            iota_t[:].rearrange("e (g d) -> e g d", d=128),
                            dstloc_t[:, k0 : k0 + G]
                            .unsqueeze(2)
                            .broadcast_to([128, G, 128]),
                            ALU.is_equal,
                        )
                        hgp2 = hgp[:].rearrange("e g h c -> e g (h c)")
                        for c in range(G):
                            st = stg[:, c * 128 : (c + 1) * 128]
                            fc = g == 0 and c == 0
                            lc = g == 1 and c == G - 1
                            nc.tensor.matmul(
                                den[:], st, pb[:, c, :], start=fc, stop=lc,
                            )
                            nc.tensor.matmul(
                                psw[:, 0:512], st, hgp2[:, c, 0:512],
                                start=fc, stop=lc,
                            )
                            nc.tensor.matmul(
                                psw[:, 512:1024], st, hgp2[:, c, 512:1024],
                                start=fc, stop=lc,
                            )
                    # window epilogue: exact self-loop term from local bf16
                    # activations, then h1 = elu(agg/denom + b1); h1T via PE
                    asw = wB.tile([128, 8], F32, tag="asw")
                    nc.vector.tensor_tensor(
                        asw[:], ssw_t[:, w, :], sdw_t[:, w, :], ALU.add
                    )
                    asl = wB.tile([128, 8], F32, tag="asl")
                    nc.vector.scalar_tensor_tensor(
                        asl[:], asw[:], SLOPE, asw[:], ALU.mult, ALU.max
                    )
                    ase = wB.tile([128, 8], F32, tag="ase")
                    nc.scalar.activation(ase[:], asl[:], ACTF.Exp)
                    dens = wB.tile([128, 8], F32, tag="dens")
                    nc.vector.tensor_tensor(dens[:], den[:], ase[:], ALU.add)
                    nc.vector.tensor_scalar(
                        dens[:], dens[:], _EPS, None, ALU.max
                    )
                    rec = wB.tile([128, 8], F32, tag="rec")
                    nc.vector.reciprocal(rec[:], dens[:])
                    h1r = wB.tile([128, D1], F32, tag="h1r")
                    nc.vector.tensor_tensor(
                        h1r[:].rearrange("e (h c) -> e h c", c=C1),
                        hloc_t[:, w, :].rearrange("e (h c) -> e h c", c=C1),
                        ase[:].unsqueeze(2).broadcast_to([128, 8, C1]),
                        ALU.mult,
                    )
                    nc.vector.tensor_tensor(h1r[:], h1r[:], psw[:], ALU.add)
                    for half in range(2):
                        o = 512 * half
                        nc.vector.tensor_tensor(
                            h1r[:, o : o + 512].rearrange(
                                "e (h c) -> e h c", c=C1
                            ),
                            h1r[:, o : o + 512].rearrange(
                                "e (h c) -> e h c", c=C1
                            ),
                            rec[:, 4 * half : 4 * half + 4]
                            .unsqueeze(2)
                            .broadcast_to([128, 4, C1]),
                            ALU.mult,
                        )
                    if add_b1:
                        nc.vector.tensor_tensor(
                            h1r[:], h1r[:], b1_t[:], ALU.add
                        )
                    etmp = wB.tile([128, D1], F32, tag="etmp")
                    nc.scalar.activation(etmp[:], h1r[:], ACTF.Exp)
                    nc.vector.tensor_scalar(
                        etmp[:], etmp[:], 1.0, 0.0, ALU.subtract, ALU.min
                    )
                    nc.vector.tensor_scalar(
                        h1r[:], h1r[:], 0.0, None, ALU.max
                    )
                    h1b = wB.tile([128, D1], BF16, tag="h1b")
                    nc.vector.tensor_tensor(h1b[:], h1r[:], etmp[:], ALU.add)
                    for j in range(8):
                        tp = den_p.tile([128, 128], BF16, tag="tp")
                        nc.tensor.transpose(
                            tp[:, 0:128], h1b[:, j * 128 : (j + 1) * 128],
                            ident_t[:],
                        )
                        nc.scalar.activation(
                            h1T_t[:, j, w * 128 : w * 128 + wl],
                            tp[:, 0:wl], ACTF.Copy,
                        )
                    # fused layer-2 GEMM for this window
                    ps2 = psC_p.tile([128, 512], F32, tag="ps2")
                    for k in range(8):
                        nc.tensor.matmul(
                            ps2[:wl, 0 : C2 + 2],
                            h1T_t[:, k, w * 128 : w * 128 + wl],
                            W2_t[:, k, :],
                            start=(k == 0), stop=(k == 7),
                        )
                    h2t = oC.tile([128, ROW2], BF16, tag="h2t")
                    if w % 5 < 2:
                        nc.vector.memset(h2t[:, 130:256], 0.0)
                    nc.vector.tensor_copy(h2t[:wl, 0:128], ps2[:wl, 0:128])
                    nc.vector.tensor_copy(
                        h2t[:wl].bitcast(F32)[:, 64:65], ps2[:wl, 128:129]
                    )
                    nc.vector.tensor_copy(
                        s2dcol_t[:wl, w : w + 1], ps2[:wl, 129:130]
                    )
                    nc.vector.tensor_copy(
                        s2scol_t[:wl, w : w + 1], ps2[:wl, 128:129]
                    )
                    nc.vector.tensor_copy(
                        h2loc_t[:wl, w, :], ps2[:wl, 0:128]
                    )
                    nc.sync.dma_start(
                        h2loc[w * 128 : w * 128 + wl, :], h2t[:wl, :]
                    )

            def _bc_scope(ws):
                with (
                    tc.tile_pool(name="edgeB", bufs=4) as eB,
                    tc.tile_pool(name="hpB", bufs=3) as hp,
                    tc.tile_pool(name="winB", bufs=2) as wB,
                    tc.tile_pool(name="outC", bufs=2) as oC,
                    tc.tile_pool(name="psw", bufs=2, space="PSUM") as psw_p,
                    tc.tile_pool(name="den", bufs=1, space="PSUM") as den_p,
                    tc.tile_pool(name="esp", bufs=1, space="PSUM") as es_p,
                    tc.tile_pool(name="psC", bufs=1, space="PSUM") as psC_p,
                ):
                    for w in ws:
                        _bc_window(
                            w, eB, hp, wB, oC, psw_p, den_p, es_p, psC_p
                        )

            if AGSPLIT:
                _bc_scope(range(5))
                nc.gpsimd.collective_compute(
                    "AllGather",
                    ALU.bypass,
                    ins=[h2loc[0:SPLIT, :]],
                    outs=[h2all[0 : NDEV * SPLIT, :]],
                    replica_groups=[list(range(NDEV))],
                )
                _bc_scope(range(5, NW))
                nc.gpsimd.collective_compute(
                    "AllGather",
                    ALU.bypass,
                    ins=[h2loc[SPLIT:NLOC, :]],
                    outs=[h2all[NDEV * SPLIT : N, :]],
                    replica_groups=[list(range(NDEV))],
                )
            else:
                _bc_scope(range(NW))
                nc.gpsimd.collective_compute(
                    "AllGather",
                    ALU.bypass,
                    ins=[h2loc[:]],
                    outs=[h2all[:]],
                    replica_groups=[list(range(NDEV))],
                )

            # ---------------- Phase D: layer-2 edge aggregation
            with (
                tc.tile_pool(name="edgeD", bufs=4) as eD,
                tc.tile_pool(name="winD", bufs=3) as wD,
                tc.tile_pool(name="psw2", bufs=2, space="PSUM") as psw2_p,
                tc.tile_pool(name="den2", bufs=1, space="PSUM") as den2_p,
                tc.tile_pool(name="esp2", bufs=1, space="PSUM") as es2_p,
            ):
                for w in range(NW):
                    wl = 128 if w < NW - 1 else LASTW
                    psw2 = psw2_p.tile([128, 128], F32, tag="psw2")
                    den2 = den2_p.tile([128, 8], F32, tag="den2")
                    dsw2 = wD.tile([128, CPW * 128], BF16, tag="dsw2")
                    nc.sync.dma_start(
                        dsw2[:],
                        dstrep_d[:, w * CPW * 128 : (w + 1) * CPW * 128],
                    )
                    for g in range(2):
                        k0 = w * CPW + g * G
                        o16 = k0 * 8
                        g2 = eD.tile([128, G, ROW2], BF16, tag="g2")
                        gh = (G + 1) // 2
                        for hv in range(2):
                            c0, c1 = hv * gh, min(G, (hv + 1) * gh)
                            if c0 >= c1:
                                continue
                            nc.gpsimd.dma_gather(
                                out_ap=g2[:, c0:c1, :], in_ap=h2all[:, :],
                                idxs_ap=srcidx2_t[:, o16 + c0 * 8 : o16 + c1 * 8],
                                num_idxs=(c1 - c0) * 128,
                                num_idxs_reg=(c1 - c0) * 128,
                                elem_size=ROW2, single_packet=True,
                            )
                        Ssb2 = wD.tile([128, G * 128], BF16, tag="Ssb2")
                        nc.vector.tensor_scalar(
                            Ssb2[:], dsw2[:, g * G * 128 : (g + 1) * G * 128],
                            iotac_t[:, 0:1], None, ALU.is_equal,
                        )
                        esp2 = es2_p.tile([128, G], F32, tag="esp2")
                        for c in range(G):
                            nc.tensor.matmul(
                                esp2[:, c : c + 1],
                                Ssb2[:, c * 128 : (c + 1) * 128],
                                s2dcol_t[:, w : w + 1],
                                start=True, stop=True,
                            )
                        pt2 = eD.tile([128, G, 1], F32, tag="pt2")
                        nc.vector.tensor_tensor(
                            pt2[:],
                            g2[:].bitcast(F32)[:, :, 64:65],
                            esp2[:].unsqueeze(2),
                            ALU.add,
                        )
                        lr2 = eD.tile([128, G, 1], F32, tag="lr2")
                        nc.vector.scalar_tensor_tensor(
                            lr2[:], pt2[:], SLOPE, pt2[:], ALU.mult, ALU.max
                        )
                        pb2 = eD.tile([128, G, 1], BF16, tag="pb2")
                        nc.scalar.activation(pb2[:], lr2[:], ACTF.Exp)
                        nc.vector.tensor_tensor(
                            g2[:, :, 0:128], g2[:, :, 0:128],
                            pb2[:].broadcast_to([128, G, 128]),
                            ALU.mult,
                        )
                        stg2 = wD.tile([128, G * 128], BF16, tag="stg2")
                        nc.vector.tensor_tensor(
                            stg2[:].rearrange("e (g d) -> e g d", d=128),
                            iota_t[:].rearrange("e (g d) -> e g d", d=128),
                            dstloc_t[:, k0 : k0 + G]
                            .unsqueeze(2)
                            .broadcast_to([128, G, 128]),
                            ALU.is_equal,
                        )
                        for c in range(G):
                            st = stg2[:, c * 128 : (c + 1) * 128]
                            fc = g == 0 and c == 0
                            lc = g == 1 and c == G - 1
                            nc.tensor.matmul(
                                den2[:, 0:1], st, pb2[:, c, :],
                                start=fc, stop=lc,
                            )
                            nc.tensor.matmul(
                                psw2[:, 0:128], st, g2[:, c, 0:128],
                                start=fc, stop=lc,
                            )
                    as2 = wD.tile([128, 1], F32, tag="as2")
                    nc.vector.tensor_tensor(
                        as2[:], s2scol_t[:, w : w + 1],
                        s2dcol_t[:, w : w + 1], ALU.add,
                    )
                    as2l = wD.tile([128, 1], F32, tag="as2l")
                    nc.vector.scalar_tensor_tensor(
                        as2l[:], as2[:], SLOPE, as2[:], ALU.mult, ALU.max
                    )
                    as2e = wD.tile([128, 1], F32, tag="as2e")
                    nc.scalar.activation(as2e[:], as2l[:], ACTF.Exp)
                    dens2 = wD.tile([128, 1], F32, tag="dens2")
                    nc.vector.tensor_tensor(
                        dens2[:], den2[:, 0:1], as2e[:], ALU.add
                    )
                    nc.vector.tensor_scalar(
                        dens2[:], dens2[:], _EPS, None, ALU.max
                    )
                    rec2 = wD.tile([128, 1], F32, tag="rec2")
                    nc.vector.reciprocal(rec2[:], dens2[:])
                    ot = wD.tile([128, C2], F32, tag="ot")
                    nc.vector.tensor_scalar(
                        ot[:], h2loc_t[:, w, :], as2e[:, 0:1], None, ALU.mult
                    )
                    nc.vector.tensor_tensor(
                        ot[:], ot[:], psw2[:, 0:128], ALU.add
                    )
                    nc.vector.tensor_scalar(
                        ot[:], ot[:], rec2[:, 0:1], None, ALU.mult
                    )
                    if add_b2:
                        nc.vector.tensor_tensor(ot[:], ot[:], b2_t[:], ALU.add)
                    nc.sync.dma_start(
                        out_d[w * 128 : w * 128 + wl, :], ot[:wl, :]
                    )

    nc.finalize()
    return nc


# ------------------------------------------------------------------ entrypoint

TRACE = [False]
LAST = [None]


def kernel(x, edge_index, W1, a_src1, a_dst1, b1, W2, a_src2, a_dst2, b2):
    x = np.asarray(x, np.float32)
    W1 = np.asarray(W1, np.float32)
    W2 = np.asarray(W2, np.float32)
    a_src1 = np.asarray(a_src1, np.float32)
    a_dst1 = np.asarray(a_dst1, np.float32)
    a_src2 = np.asarray(a_src2, np.float32)
    a_dst2 = np.asarray(a_dst2, np.float32)
    b1 = np.asarray(b1, np.float32)
    b2 = np.asarray(b2, np.float32)
    ei = np.asarray(edge_index)

    devs, G, CPW = _prep_edges(ei)

    # fold attention projections into the GEMM weights
    A1 = np.zeros((D1, 16), np.float32)
    for h in range(H1):
        A1[h * C1 : (h + 1) * C1, h] = a_src1[h]
        A1[h * C1 : (h + 1) * C1, 8 + h] = a_dst1[h]

    W1aug = np.concatenate([W1, W1 @ A1], axis=1)
    W2aug = np.concatenate(
        [W2, W2 @ a_src2[0][:, None], W2 @ a_dst2[0][:, None]], 1
    )
    add_b1 = bool(np.any(b1 != 0))
    add_b2 = bool(np.any(b2 != 0))

    key = (G, CPW, add_b1, add_b2, AGSPLIT)
    if key not in _CACHE:
        _CACHE[key] = _build(G, CPW, add_b1, add_b2)
    nc = _CACHE[key]

    xT = x.T
    shared = {
        "W1aug": _bf(W1aug),
        "W2aug": _bf(W2aug),
        "iota_rep": _bf(
            np.tile(np.arange(128, dtype=np.float32)[None, :], (128, G))
        ),
        "iota_col": np.arange(128, dtype=np.float32)[:, None].copy(),
        "ident": _bf(np.eye(128, dtype=np.float32)),
        "b1bc": np.ascontiguousarray(np.tile(b1[None, :], (128, 1))),
        "b2bc": np.ascontiguousarray(np.tile(b2[None, :], (128, 1))),
    }
    in_maps = []
    for d in range(NDEV):
        m = {**shared, **devs[d]}
        m["xT"] = _bf(np.roll(xT, -d * NLOC, axis=1))
        in_maps.append(m)

    if os.environ.get("KSIM"):
        from concourse.bass_interp import MultiCoreSim

        sim = MultiCoreSim(
            nc,
            num_cores=NDEV,
            num_workers=int(os.environ.get("KSIM_WORKERS", "8")),
            require_finite=False,
            require_nnan=False,
        )
        for d in range(NDEV):
            cs = sim.cores[d]
            for k2, v in in_maps[d].items():
                cs.tensor(k2)[:] = v
        sim.simulate(check_with_hw=False)
        out = np.concatenate(
            [np.array(sim.cores[d].tensor("out")) for d in range(NDEV)], axis=0
        )
        LAST[0] = None
        return out.astype(np.float32)

    res = run_bass_kernel_spmd(nc, in_maps, list(range(NDEV)), trace=TRACE[0])
    LAST[0] = res
    out = np.concatenate([res.results[d]["out"] for d in range(NDEV)], axis=0)
    return out.astype(np.float32)
